# revision 1
# baseline (speedup 1.0000x reference)
"""MoE routing kernel (nn_JSMLP): per-row expert-indexed 3-layer MLP.

  out[n] = Wl[i] @ tanh(W2[i] @ tanh(W1[i] @ x[n] + b1[i]) + b2[i]) + bl[i],  i = ind[n]

Sharding strategy (hardcoded): expert-parallel across the 8 cores.
Host groups rows by expert (argsort of ind), pads each expert's rows to a
uniform capacity C, and assigns 32 consecutive experts to each core. Each
core then runs dense per-expert GEMMs in a transposed layout (hidden dim on
SBUF partitions, rows on the free dim), in bf16 with fp32 PSUM accumulation:

  L1: H1ᵀ[256, C] = W1augᵀ.T @ xaugᵀ      (bias via ones-row augmentation, K=65)
  L2: H2ᵀ[256, C] = W2ᵀ.T @ tanh(H1ᵀ)     (K=256 split in 2, bias via a tiny
                                           block-diagonal ones matmul)
  L3: outᵀ[64, C] = Wlᵀ.T @ tanh(H2ᵀ)     (two experts packed per 128-partition
                                           PSUM tile, bias as in L2)

tanh runs on ScalarE over wide multi-expert spans to amortize the per-op
overhead. Everything is statically compiled after inspecting the routing
(capacity C is derived from the actual max expert load), so the single SPMD
program is identical across cores and only the per-core data differs.
"""

import numpy as np
import ml_dtypes

N, IN_DIM, H1, H2, LIN, NEXP = 16384, 64, 256, 256, 64, 256
NCORES = 8
EPC = NEXP // NCORES  # experts per core

BF16 = ml_dtypes.bfloat16

_cache = {}


def _build_program(C, B):
    """Build the SPMD Bass program for capacity C with B experts per block."""
    import concourse.bass as bass
    import concourse.tile as tile
    from concourse import bacc, mybir

    S = B * C            # columns of one block's expert group
    ncb = B // 2         # column blocks in the packed L3 output
    blocks = EPC // B
    f32 = mybir.dt.float32
    bf16 = mybir.dt.bfloat16
    Tanh = mybir.ActivationFunctionType.Tanh

    nc = bacc.Bacc("TRN2", target_bir_lowering=False, debug=False,
                   num_devices=NCORES)

    # DMA count is the scarce resource (HWDGE descriptor generation is a
    # shared serial device, ~625ns per dma_start), so tensors are merged into
    # one [65, .]-partition load (x + W1) and one [128, .] load (W2/Wl
    # K-chunks) per block, plus a single constants load.
    XW = B * (C + 256)               # xg cols [0, B*C), w1t cols [B*C, XW)
    WB = B * 640                     # w2a | w2b | wla | wlb sections
    O_W2B, O_WLA, O_WLB = B * 256, 2 * B * 256, 2 * B * 256 + B * 64
    CT = blocks * 256 + blocks * 128 + S   # w2c | wlc | bdiag sections
    O_WLC, O_BD = blocks * 256, blocks * 256 + blocks * 128

    xw_d = nc.dram_tensor("xw", [blocks, 65, XW], bf16, kind="ExternalInput")
    wb_d = nc.dram_tensor("wb", [blocks, 128, WB], bf16, kind="ExternalInput")
    ct_d = nc.dram_tensor("ct", [B, CT], bf16, kind="ExternalInput")
    out_d = nc.dram_tensor("out", [128, blocks * ncb * C], bf16,
                           kind="ExternalOutput")

    with tile.TileContext(nc) as tc:
        with (
            tc.tile_pool(name="consts", bufs=1) as cpool,
            tc.tile_pool(name="wts", bufs=8) as wpool,
            tc.tile_pool(name="acts", bufs=8) as hpool,
            tc.tile_pool(name="ostage", bufs=1) as opool,
            tc.tile_pool(name="ph", bufs=2, space=bass.MemorySpace.PSUM) as phpool,
            tc.tile_pool(name="po", bufs=4, space=bass.MemorySpace.PSUM) as popool,
        ):
            ct = cpool.tile([B, CT], bf16, tag="ct")
            nc.sync.dma_start(ct[:], ct_d.ap())
            w2ct = ct[:, 0:O_WLC]
            wlct = ct[0:ncb, O_WLC:O_BD]
            bdt = ct[:, O_BD:O_BD + S]
            # bf16 staging/store: halves output bytes on the serial DMA pipe;
            # the DVE copy below does the fp32->bf16 cast for free
            ostage = opool.tile([128, blocks * ncb * C], bf16, tag="out")

            for b in range(blocks):
                xwt = wpool.tile([65, XW], bf16, tag="xw")
                nc.sync.dma_start(xwt[:], xw_d.ap()[b])
                xgt = xwt[:, 0:B * C]
                w1tt = xwt[:, B * C:XW]
                wbt = wpool.tile([128, WB], bf16, tag="wb")
                nc.sync.dma_start(wbt[:], wb_d.ap()[b])
                w2at = wbt[:, 0:O_W2B]
                w2bt = wbt[:, O_W2B:O_WLA]
                wlat = wbt[:, O_WLA:O_WLB]
                wlbt = wbt[:, O_WLB:WB]

                # L1: H1preT[256, S] — hidden half t lives at cols [t*512, t*512+S)
                # (512-aligned so no matmul output crosses a PSUM bank boundary).
                ph1 = phpool.tile([128, 1024], f32, tag="ph")
                for j in range(B):
                    for t in range(2):
                        nc.tensor.matmul(
                            ph1[:, t * 512 + j * C : t * 512 + (j + 1) * C],
                            w1tt[:, j * 256 + t * 128 : j * 256 + (t + 1) * 128],
                            xgt[:, j * C : (j + 1) * C],
                        )
                h1 = hpool.tile([128, 2 * S], bf16, tag="h1")
                nc.scalar.activation(
                    h1[:].rearrange("p (t s) -> p t s", t=2),
                    ph1[:].rearrange("p (t s) -> p t s", t=2)[:, :, 0:S],
                    Tanh,
                )

                # L2: bias seeded over the whole span, then 2 accumulating
                # K-chunks per expert.
                ph2 = phpool.tile([128, 1024], f32, tag="ph")
                for t in range(2):
                    nc.tensor.matmul(
                        ph2[:, t * 512 : t * 512 + S],
                        w2ct[:, (b * 2 + t) * 128 : (b * 2 + t + 1) * 128],
                        bdt[:, 0:S],
                        start=True, stop=False, skip_group_check=True,
                    )
                    for j in range(B):
                        nc.tensor.matmul(
                            ph2[:, t * 512 + j * C : t * 512 + (j + 1) * C],
                            w2at[:, j * 256 + t * 128 : j * 256 + (t + 1) * 128],
                            h1[:, j * C : (j + 1) * C],
                            start=False, stop=False, skip_group_check=True,
                        )
                        nc.tensor.matmul(
                            ph2[:, t * 512 + j * C : t * 512 + (j + 1) * C],
                            w2bt[:, j * 256 + t * 128 : j * 256 + (t + 1) * 128],
                            h1[:, S + j * C : S + (j + 1) * C],
                            start=False, stop=True, skip_group_check=True,
                        )
                h2 = hpool.tile([128, 2 * S], bf16, tag="h2")
                nc.scalar.activation(
                    h2[:].rearrange("p (t s) -> p t s", t=2),
                    ph2[:].rearrange("p (t s) -> p t s", t=2)[:, :, 0:S],
                    Tanh,
                )

                # L3: experts packed 2-per-partition-block: expert j -> output
                # partitions [64*(j%2), +64), columns [(j//2)*C, +C).
                po = popool.tile([128, ncb * C], f32, tag="po")
                for h in range(2):
                    nc.tensor.matmul(
                        po[h * 64 : (h + 1) * 64, :],
                        wlct[:, (b * 2 + h) * 64 : (b * 2 + h + 1) * 64],
                        bdt[0:ncb, 0 : ncb * C],
                        start=True, stop=False, skip_group_check=True,
                    )
                for j in range(B):
                    h_, cb = j % 2, j // 2
                    nc.tensor.matmul(
                        po[h_ * 64 : (h_ + 1) * 64, cb * C : (cb + 1) * C],
                        wlat[:, j * 64 : (j + 1) * 64],
                        h2[:, j * C : (j + 1) * C],
                        start=False, stop=False, skip_group_check=True,
                    )
                    nc.tensor.matmul(
                        po[h_ * 64 : (h_ + 1) * 64, cb * C : (cb + 1) * C],
                        wlbt[:, j * 64 : (j + 1) * 64],
                        h2[:, S + j * C : S + (j + 1) * C],
                        start=False, stop=True, skip_group_check=True,
                    )
                nc.vector.tensor_copy(
                    ostage[:, b * ncb * C : (b + 1) * ncb * C], po[:]
                )
                if b % 2 == 1:
                    # store from the ACT-engine HWDGE queue so input loads on
                    # the SP queue never queue behind a compute-gated store
                    nc.scalar.dma_start(
                        out_d.ap()[:, (b - 1) * ncb * C : (b + 1) * ncb * C],
                        ostage[:, (b - 1) * ncb * C : (b + 1) * ncb * C],
                    )

    nc.compile()
    return nc


def _prep_inputs(x, ind, W1, b1, W2, b2, Wl, bl, C, B):
    """Group rows by expert and build the per-core padded device arrays."""
    blocks = EPC // B
    ncb = B // 2
    S = B * C

    order = np.argsort(ind, kind="stable")
    counts = np.bincount(ind, minlength=NEXP)
    offs = np.zeros(NEXP + 1, np.int64)
    np.cumsum(counts, out=offs[1:])
    rows = [order[offs[e]:offs[e + 1]] for e in range(NEXP)]

    # Augmented, transposed weight tables (built once across all cores).
    # W1augT[e] = [65, 256]: rows 0:64 = W1[e].T, row 64 = b1[e].
    w1aug = np.concatenate([W1, b1[:, :, None]], axis=2)  # [E, 256, 65]
    w2aug = np.concatenate([W2, b2[:, :, None]], axis=2)  # [E, 256, 257]
    wlaug = np.concatenate([Wl, bl[:, :, None]], axis=2)  # [E, 64, 257]

    XW = B * (C + 256)
    WB = B * 640
    O_W2B, O_WLA, O_WLB = B * 256, 2 * B * 256, 2 * B * 256 + B * 64
    CT = blocks * 256 + blocks * 128 + S
    O_WLC, O_BD = blocks * 256, blocks * 256 + blocks * 128

    in_maps = []
    for k in range(NCORES):
        es = np.arange(k * EPC, (k + 1) * EPC)
        xw = np.zeros((blocks, 65, XW), np.float32)
        xw[:, 64, 0:B * C] = 1.0  # ones row of the augmented x
        wb = np.empty((blocks, 128, WB), np.float32)
        ct = np.zeros((B, CT), np.float32)
        for b in range(blocks):
            for j in range(B):
                e = es[b * B + j]
                r = rows[e]
                xw[b, 0:64, j * C : j * C + len(r)] = x[r].T
                xw[b, :, B * C + j * 256 : B * C + (j + 1) * 256] = w1aug[e].T
                wb[b, :, j * 256 : (j + 1) * 256] = w2aug[e, :, 0:128].T
                wb[b, :, O_W2B + j * 256 : O_W2B + (j + 1) * 256] = \
                    w2aug[e, :, 128:256].T
                wb[b, :, O_WLA + j * 64 : O_WLA + (j + 1) * 64] = \
                    wlaug[e, :, 0:128].T
                wb[b, :, O_WLB + j * 64 : O_WLB + (j + 1) * 64] = \
                    wlaug[e, :, 128:256].T
                # w2c[j, b, t] = b2-augmented row of expert e, chunk t
                ct[j, b * 256 : (b + 1) * 256] = w2aug[e, :, 256]
            # wlc[cb, b, h] = bl-augmented row of expert B*b + 2*cb + h
            for cb in range(ncb):
                for h in range(2):
                    e = es[b * B + 2 * cb + h]
                    ct[cb, O_WLC + b * 128 + h * 64 : O_WLC + b * 128 + (h + 1) * 64] = \
                        wlaug[e, :, 256]
        for j in range(B):
            ct[j, O_BD + j * C : O_BD + (j + 1) * C] = 1.0
        in_maps.append({
            "xw": xw.astype(BF16),
            "wb": wb.astype(BF16),
            "ct": ct.astype(BF16),
        })
    return in_maps, rows


def _unscatter(results, rows, C, B):
    blocks = EPC // B
    ncb = B // 2
    out = np.empty((N, LIN), np.float32)
    for k in range(NCORES):
        arr = np.asarray(results[k]["out"], np.float32).reshape(2, 64, blocks, ncb, C)
        for b in range(blocks):
            for cb in range(ncb):
                for h in range(2):
                    e = k * EPC + b * B + 2 * cb + h
                    r = rows[e]
                    out[r, :] = arr[h, :, b, cb, 0:len(r)].T
    return out


def kernel(x, ind, W1, b1, W2, b2, Wl, bl):
    from concourse.bass_utils import run_bass_kernel_spmd

    x = np.asarray(x, np.float32)
    ind = np.asarray(ind).astype(np.int64)
    W1 = np.asarray(W1, np.float32); b1 = np.asarray(b1, np.float32)
    W2 = np.asarray(W2, np.float32); b2 = np.asarray(b2, np.float32)
    Wl = np.asarray(Wl, np.float32); bl = np.asarray(bl, np.float32)

    counts = np.bincount(ind, minlength=NEXP)
    C = max(32, int(np.ceil(counts.max() / 32)) * 32)
    assert C <= 256, f"expert load {counts.max()} too imbalanced for this kernel"
    B = 4 if C <= 128 else 2  # keep B*C <= 512 (one PSUM bank per block span)

    key = (C, B)
    if key not in _cache:
        _cache[key] = _build_program(C, B)
    nc = _cache[key]

    in_maps, rows = _prep_inputs(x, ind, W1, b1, W2, b2, Wl, bl, C, B)
    res = run_bass_kernel_spmd(nc, in_maps, core_ids=list(range(NCORES)))
    return _unscatter(res.results, rows, C, B)



# revision 7
# speedup vs baseline: 1.4545x; 1.4545x over previous
"""MoE routing kernel (nn_JSMLP): per-row expert-indexed 3-layer MLP.

  out[n] = Wl[i] @ tanh(W2[i] @ tanh(W1[i] @ x[n] + b1[i]) + b2[i]) + bl[i],  i = ind[n]

Sharding (hardcoded): expert-parallel across 8 cores, load-balanced by
count-sorted round-robin so a single SPMD program fits all cores:
experts are sorted by row count (desc); rank r goes to core r%8, slot r//8.
Slot s then has the same capacity cap[s] = roundup(count of rank 8s, 4) on
every core, so per-slot column spans are compile-time constants while padding
stays ~3% (vs ~50% for a uniform max-count capacity).

Numerics: W1 (with b1 row) and W2 are stored as float8_e3m4 scaled by 64
(values land in e3m4's normal range; 4 mantissa bits ~ 1.2% rms/elem); the
1/64 descale rides the tanh activations for free (out = tanh(scale*in)).
Wl, x, h, biases stay bf16; PSUM accumulates fp32. Measured end-to-end rel
err ~1.7e-2 vs the fp32 reference (gate: 2e-2).

Per core, per block b (4 slots, S_b = sum of caps <= 512 = one PSUM bank):
  L1: H1T[256, S_b] = W1augT.T @ [x;1]T   (bias via ones-row, K=65, e3m4 x64)
  L2: H2T[256, S_b] = W2T.T @ tanh(H1T/64)  (K=256 in 2 chunks; b2 seeded by a
      tiny block-diag ones matmul in bf16, also x64)
  L3: outT[2x64, PS_b] = WlT.T @ tanh(H2T/64)  (2 experts per 128-partition
      tile; bl seeded via pair-diag ones; plain bf16)

DMA plan (the shared DMA engine pool is the roofline at ~360 B/ns): one load
each for consts/x/W1 (W1 split in 2), per-2-block loads for W2/Wl, per-2-block
stores on the DVE queue; 15 DMAs total keeps the serial HWDGE (~630ns each)
off the critical path. All loads are issued into resident SBUF tiles (no ring
reuse) so the load queue never blocks on consumers.
"""

import numpy as np
import ml_dtypes

N, IN_DIM, H1, H2, LIN, NEXP = 16384, 64, 256, 256, 64, 256
NCORES = 8
SLOTS = NEXP // NCORES  # 32 experts per core

BF16 = ml_dtypes.bfloat16
E3M4 = ml_dtypes.float8_e3m4
WSCALE = 64.0

_cache = {}


def _geometry(caps):
    """Block/pair geometry shared by program builder and host prep.

    caps: per-slot capacities (len 32, multiples of 4, may be 0).
    Returns dict with blocks (list of slot-index lists), per-block slot
    offsets, block x-offsets, pair layout and output offsets.
    """
    blocks = []
    cur, cur_sum = [], 0
    for s in range(SLOTS):
        c = caps[s]
        if c == 0:
            continue
        if len(cur) == 4 or (cur_sum + c > 512 and cur):
            blocks.append(cur)
            cur, cur_sum = [], 0
        cur.append(s)
        cur_sum += c
    if cur:
        blocks.append(cur)

    g = {"blocks": blocks, "xoff": [], "S": [], "XO": [], "pairs": [],
         "poff": [], "PS": [], "OO": []}
    xo_total, oo_total = 0, 0
    for bl in blocks:
        offs, acc = [], 0
        for s in bl:
            offs.append(acc)
            acc += caps[s]
        assert acc <= 512, f"block span {acc} exceeds a PSUM bank"
        g["xoff"].append(offs)
        g["S"].append(acc)
        g["XO"].append(xo_total)
        xo_total += acc
        prs = [(bl[i], bl[i + 1] if i + 1 < len(bl) else None)
               for i in range(0, len(bl), 2)]
        poffs, pacc = [], 0
        for a, b in prs:
            poffs.append(pacc)
            pacc += max(caps[a], caps[b] if b is not None else 0)
        g["pairs"].append(prs)
        g["poff"].append(poffs)
        g["PS"].append(pacc)
        g["OO"].append(oo_total)
        oo_total += pacc
    g["TOT"] = xo_total
    g["TOT2"] = oo_total
    return g


def _build_program(caps):
    import concourse.bass as bass
    import concourse.tile as tile
    from concourse import bacc, mybir

    caps = list(caps)
    g = _geometry(caps)
    blocks, S, XO, xoff = g["blocks"], g["S"], g["XO"], g["xoff"]
    pairs, poff, PS, OO = g["pairs"], g["poff"], g["PS"], g["OO"]
    NB = len(blocks)
    TOT, TOT2 = g["TOT"], g["TOT2"]

    f32 = mybir.dt.float32
    bf16 = mybir.dt.bfloat16
    e3 = mybir.dt.float8e3
    Tanh = mybir.ActivationFunctionType.Tanh

    # ct columns: [w2c: NB*256 | wlc: NB*128 (rows 0:2) | bdt: TOT | bdl: TOT2]
    O_WLC = NB * 256
    O_BDT = O_WLC + NB * 128
    O_BDL = O_BDT + TOT
    CTW = O_BDL + TOT2

    nc = bacc.Bacc("TRN2", target_bir_lowering=False, debug=False,
                   num_devices=NCORES)

    xg_d = nc.dram_tensor("xg", [65, TOT], bf16, kind="ExternalInput")
    w1_d = nc.dram_tensor("w1", [65, SLOTS * 256], e3, kind="ExternalInput")
    w2_d = nc.dram_tensor("w2", [128, SLOTS * 512], e3, kind="ExternalInput")
    wl_d = nc.dram_tensor("wl", [128, SLOTS * 128], bf16, kind="ExternalInput")
    ct_d = nc.dram_tensor("ct", [4, CTW], bf16, kind="ExternalInput")
    out_d = nc.dram_tensor("out", [128, TOT2], bf16, kind="ExternalOutput")

    # block -> first/last slot columns for the per-2-block weight loads
    def slot_range(b0, b1):
        lo = blocks[b0][0]
        hi = blocks[b1][-1] + 1
        return lo, hi

    with tile.TileContext(nc) as tc:
        with (
            tc.tile_pool(name="stat", bufs=1) as spool,
            tc.tile_pool(name="acts", bufs=NB) as hpool,
            tc.tile_pool(name="ph", bufs=3, space=bass.MemorySpace.PSUM) as php,
            tc.tile_pool(name="po", bufs=2, space=bass.MemorySpace.PSUM) as pop,
        ):
            ct = spool.tile([4, CTW], bf16, tag="ct")
            xg = spool.tile([65, TOT], bf16, tag="xg")
            w1t = spool.tile([65, SLOTS * 256], e3, tag="w1")
            w2t = spool.tile([128, SLOTS * 512], e3, tag="w2")
            wlt = spool.tile([128, SLOTS * 128], bf16, tag="wl")
            ostage = spool.tile([128, TOT2], bf16, tag="out")

            nc.sync.dma_start(ct[:], ct_d.ap())
            nc.sync.dma_start(xg[:], xg_d.ap())
            # W1 split in two so block 0's L1 isn't gated on the full tensor
            half = (SLOTS // 2) * 256
            nc.sync.dma_start(w1t[:, 0:half], w1_d.ap()[:, 0:half])
            nc.sync.dma_start(w1t[:, half:], w1_d.ap()[:, half:])
            for b0 in range(0, NB, 2):
                b1 = min(b0 + 1, NB - 1)
                lo, hi = slot_range(b0, b1)
                nc.sync.dma_start(w2t[:, lo * 512:hi * 512],
                                  w2_d.ap()[:, lo * 512:hi * 512])
                nc.sync.dma_start(wlt[:, lo * 128:hi * 128],
                                  wl_d.ap()[:, lo * 128:hi * 128])

            for b in range(NB):
                bslots = blocks[b]
                Sb, xob = S[b], xoff[b]

                # L1: hidden half t lives at PSUM cols [t*512, t*512+Sb)
                ph1 = php.tile([128, 1024], f32, tag="ph")
                for t in range(2):
                    for i, s in enumerate(bslots):
                        c = caps[s]
                        nc.tensor.matmul(
                            ph1[:, t * 512 + xob[i]: t * 512 + xob[i] + c],
                            w1t[:, s * 256 + t * 128: s * 256 + (t + 1) * 128],
                            xg[:, XO[b] + xob[i]: XO[b] + xob[i] + c],
                        )
                h1 = hpool.tile([128, 2 * Sb], bf16, tag="h1",
                                padded_shape=[128, 2 * max(S)])
                nc.scalar.activation(
                    h1[:].rearrange("p (t s) -> p t s", t=2),
                    ph1[:].rearrange("p (t s) -> p t s", t=2)[:, :, 0:Sb],
                    Tanh, scale=1.0 / WSCALE,
                )

                # L2: b2 seeded over the span, then 2 accumulating K-chunks
                ph2 = php.tile([128, 1024], f32, tag="ph")
                for t in range(2):
                    nc.tensor.matmul(
                        ph2[:, t * 512: t * 512 + Sb],
                        ct[:, (b * 2 + t) * 128: (b * 2 + t + 1) * 128],
                        ct[:, O_BDT + XO[b]: O_BDT + XO[b] + Sb],
                        start=True, stop=False, skip_group_check=True,
                    )
                    for i, s in enumerate(bslots):
                        c = caps[s]
                        last = i == len(bslots) - 1
                        nc.tensor.matmul(
                            ph2[:, t * 512 + xob[i]: t * 512 + xob[i] + c],
                            w2t[:, s * 512 + t * 128: s * 512 + (t + 1) * 128],
                            h1[:, xob[i]: xob[i] + c],
                            start=False, stop=False, skip_group_check=True,
                        )
                        nc.tensor.matmul(
                            ph2[:, t * 512 + xob[i]: t * 512 + xob[i] + c],
                            w2t[:, s * 512 + 256 + t * 128: s * 512 + 256 + (t + 1) * 128],
                            h1[:, Sb + xob[i]: Sb + xob[i] + c],
                            start=False, stop=last, skip_group_check=True,
                        )
                h2 = hpool.tile([128, 2 * Sb], bf16, tag="h2",
                                padded_shape=[128, 2 * max(S)])
                nc.scalar.activation(
                    h2[:].rearrange("p (t s) -> p t s", t=2),
                    ph2[:].rearrange("p (t s) -> p t s", t=2)[:, :, 0:Sb],
                    Tanh, scale=1.0 / WSCALE,
                )

                # L3: 2 experts per 128-partition tile; bias via pair-diag ones
                po = pop.tile([128, PS[b]], f32, tag="po",
                              padded_shape=[128, max(PS)])
                for h in range(2):
                    nc.tensor.matmul(
                        po[h * 64:(h + 1) * 64, 0:PS[b]],
                        ct[0:2, O_WLC + b * 128 + h * 64: O_WLC + b * 128 + (h + 1) * 64],
                        ct[0:2, O_BDL + OO[b]: O_BDL + OO[b] + PS[b]],
                        start=True, stop=False, skip_group_check=True,
                    )
                nmm = sum(1 for pr in pairs[b] for s in pr if s is not None)
                k = 0
                for ci, (sa, sb_) in enumerate(pairs[b]):
                    for h, s in enumerate((sa, sb_)):
                        if s is None:
                            continue
                        c = caps[s]
                        xo = xob[2 * ci + h]
                        k += 1
                        nc.tensor.matmul(
                            po[h * 64:(h + 1) * 64,
                               poff[b][ci]: poff[b][ci] + c],
                            wlt[:, s * 128: s * 128 + 64],
                            h2[:, xo: xo + c],
                            start=False, stop=False, skip_group_check=True,
                        )
                        nc.tensor.matmul(
                            po[h * 64:(h + 1) * 64,
                               poff[b][ci]: poff[b][ci] + c],
                            wlt[:, s * 128 + 64: s * 128 + 128],
                            h2[:, Sb + xo: Sb + xo + c],
                            start=False, stop=k == nmm, skip_group_check=True,
                        )
                nc.vector.tensor_copy(ostage[:, OO[b]: OO[b] + PS[b]],
                                      po[:, 0:PS[b]])
                if b % 2 == 1 or b == NB - 1:
                    b0 = b - 1 if b % 2 == 1 else b
                    nc.sync.dma_start(
                        out_d.ap()[:, OO[b0]: OO[b] + PS[b]],
                        ostage[:, OO[b0]: OO[b] + PS[b]],
                    )

    nc.compile()
    return nc


def _plan(ind):
    counts = np.bincount(ind, minlength=NEXP)
    perm = np.argsort(-counts, kind="stable")
    caps = []
    for s in range(SLOTS):
        c = int(counts[perm[8 * s]])
        caps.append(0 if c == 0 else int(np.ceil(c / 4)) * 4)
    return counts, perm, caps


def _prep_inputs(x, ind, W1, b1, W2, b2, Wl, bl, perm, caps, g):
    """Build per-core arrays for the count-sorted round-robin layout."""
    blocks, S, XO, xoff = g["blocks"], g["S"], g["XO"], g["xoff"]
    pairs, poff, OO = g["pairs"], g["poff"], g["OO"]
    NB = len(blocks)
    TOT, TOT2 = g["TOT"], g["TOT2"]
    O_WLC = NB * 256
    O_BDT = O_WLC + NB * 128
    O_BDL = O_BDT + TOT
    CTW = O_BDL + TOT2

    order = np.argsort(ind, kind="stable")
    offs = np.zeros(NEXP + 1, np.int64)
    np.cumsum(np.bincount(ind, minlength=NEXP), out=offs[1:])
    rows = [order[offs[e]:offs[e + 1]] for e in range(NEXP)]

    # scaled transposed weights, shared across cores
    w1aug = np.concatenate([W1, b1[:, :, None]], axis=2)       # [E, 256, 65]
    w1q = (w1aug * WSCALE).astype(E3M4)                        # e3m4 x64
    w2q = (W2 * WSCALE).astype(E3M4)                           # [E, 256, 256]
    wlb = Wl.astype(BF16)                                      # [E, 64, 256]
    b2q = (b2 * WSCALE).astype(np.float32)
    xb = x.astype(BF16)

    in_maps = []
    for k in range(NCORES):
        xg = np.zeros((65, TOT), np.float32)
        w1 = np.zeros((65, SLOTS * 256), E3M4)
        w2 = np.zeros((128, SLOTS * 512), E3M4)
        wl = np.zeros((128, SLOTS * 128), np.float32)
        ct = np.zeros((4, CTW), np.float32)
        for b in range(NB):
            for i, s in enumerate(blocks[b]):
                if caps[s] == 0:
                    continue
                e = perm[8 * s + k]
                r = rows[e]
                col = XO[b] + xoff[b][i]
                xg[0:64, col: col + len(r)] = xb[r].astype(np.float32).T
                xg[64, col: col + caps[s]] = 1.0
                w1[:, s * 256:(s + 1) * 256] = w1q[e].T
                # W2 chunks A|B, each [128, 256]
                w2[:, s * 512: s * 512 + 256] = w2q[e, :, 0:128].T
                w2[:, s * 512 + 256: (s + 1) * 512] = w2q[e, :, 128:256].T
                wl[:, s * 128: s * 128 + 64] = wlb[e, :, 0:128].astype(np.float32).T
                wl[:, s * 128 + 64: (s + 1) * 128] = wlb[e, :, 128:256].astype(np.float32).T
                ct[i, b * 256:(b + 1) * 256] = b2q[e]
                ct[i, O_BDT + col: O_BDT + col + caps[s]] = 1.0
            for ci, (sa, sb_) in enumerate(pairs[b]):
                pc = OO[b] + poff[b][ci]
                w = max(caps[sa], caps[sb_] if sb_ is not None else 0)
                ct[ci, O_BDL + pc: O_BDL + pc + w] = 1.0
                for h, s in enumerate((sa, sb_)):
                    if s is None or caps[s] == 0:
                        continue
                    e = perm[8 * s + k]
                    ct[ci, O_WLC + b * 128 + h * 64: O_WLC + b * 128 + (h + 1) * 64] = bl[e]
        in_maps.append({
            "xg": xg.astype(BF16),
            "w1": w1,
            "w2": w2,
            "wl": wl.astype(BF16),
            "ct": ct.astype(BF16),
        })
    return in_maps, rows


def _unscatter(results, rows, perm, caps, g):
    blocks, xoff, poff, OO, pairs = g["blocks"], g["xoff"], g["poff"], g["OO"], g["pairs"]
    out = np.empty((N, LIN), np.float32)
    for k in range(NCORES):
        arr = np.asarray(results[k]["out"], np.float32)
        for b in range(len(blocks)):
            for ci, (sa, sb_) in enumerate(pairs[b]):
                for h, s in enumerate((sa, sb_)):
                    if s is None or caps[s] == 0:
                        continue
                    e = perm[8 * s + k]
                    r = rows[e]
                    col = OO[b] + poff[b][ci]
                    out[r, :] = arr[h * 64:(h + 1) * 64, col: col + len(r)].T
    return out


def kernel(x, ind, W1, b1, W2, b2, Wl, bl):
    from concourse.bass_utils import run_bass_kernel_spmd

    x = np.asarray(x, np.float32)
    ind = np.asarray(ind).astype(np.int64)
    W1 = np.asarray(W1, np.float32); b1 = np.asarray(b1, np.float32)
    W2 = np.asarray(W2, np.float32); b2 = np.asarray(b2, np.float32)
    Wl = np.asarray(Wl, np.float32); bl = np.asarray(bl, np.float32)

    counts, perm, caps = _plan(ind)
    g = _geometry(caps)

    key = tuple(caps)
    if key not in _cache:
        _cache[key] = _build_program(caps)
    nc = _cache[key]

    in_maps, rows = _prep_inputs(x, ind, W1, b1, W2, b2, Wl, bl, perm, caps, g)
    res = run_bass_kernel_spmd(nc, in_maps, core_ids=list(range(NCORES)))
    return _unscatter(res.results, rows, perm, caps, g)
